# revision 7
# baseline (speedup 1.0000x reference)
"""GCN encoder (GCNConv + PReLU) distributed Bass kernel for 8 TRN2 NeuronCores.

Reference computation:
    src/dst = edge_index with self loops appended
    deg[v]  = #edges with dst==v (incl. self loop)
    dinv    = rsqrt(deg)
    h       = x @ W
    agg[v]  = sum_{e: dst=v} dinv[src_e]*dinv[v]*h[src_e] + b
    out     = prelu(agg, alpha)

Factored form used on device:
    g[u]    = dinv[u] * h[u]
    out[v]  = prelu(dinv[v] * sum_{e: dst=v} g[src_e] + b)   (self loop = an edge)

Distribution: dst-node ownership sharded over 8 cores. Each core computes its
g shard (x @ W via PE transpose + matmul, scaled by rsqrt(deg)), an AllGather
replicates g (bf16), then each core gathers the 256B g rows of its edges with
dma_gather and segment-sums them on the TensorEngine: for each 128-edge block,
matmul(lhsT=g_rows[128e,128f], rhs=onehot[128e,128dst]) accumulates into a
PSUM [feat, 512dst] window. onehot = is_equal(dstrel, iota) built on VectorE.
Epilogue applies dinv[dst] (free-dim broadcast), bias and PReLU, transposes
back via PE, and writes natural-layout rows.

Host side does only integer index work: self loops, bincount, edge binning
into (window, subwindow, table-chunk) groups padded to 128-edge blocks, and
int16 gather-index tables (dma_gather needs idx<32768, so g is addressed in
<=32767-row chunks; edges are grouped by chunk).
"""

import math
import os

import numpy as np
import ml_dtypes

import concourse.bass as bass
import concourse.tile as tile
import concourse.bacc as bacc
from concourse import mybir
from concourse.bass_utils import run_bass_kernel_spmd
from concourse.masks import make_identity

N_CORES = 8
P = 128          # partitions / feature dim
WIN = 512        # dst columns per PSUM window (one f32 bank)
SUB = 128        # dst columns per one-hot subwindow
NSUB = WIN // SUB
WPC = int(os.environ.get("GCN_WPC", "1"))  # windows per dma_gather call group
NQ = int(os.environ.get("GCN_NQ", "4"))    # SWDGE queues (gather desc-gen core pairs)
MAX_CHUNK = int(os.environ.get("GCN_MAXCHUNK", "32767"))  # dma_gather int16 idx limit
SORT_SRC = os.environ.get("GCN_SORT", "0") == "1"  # sort gathers by source within group
PAD_MODE = os.environ.get("GCN_PAD", "spread")     # pad idx: "zero" | "spread" | "ffill"
SINGLE_PACKET = os.environ.get("GCN_SP", "0") == "1"

BF16 = mybir.dt.bfloat16
F32 = mybir.dt.float32
I16 = mybir.dt.int16


def _wrap_idx16(flat):
    """[n] int -> [128, n//16] int16: idx i at partition i%16 position i//16,
    replicated to all 8 Q7 core groups."""
    n = flat.shape[0]
    a = flat.reshape(n // 16, 16).T.astype(np.int16)      # [16, n//16]
    return np.tile(a, (8, 1))                              # [128, n//16]


def _host_prep(x, edge_index, W, b, alpha):
    N, D = x.shape
    assert D == P and N % N_CORES == 0
    npc = N // N_CORES
    npc_pad = ((npc + P - 1) // P) * P
    nwin = (npc + WIN - 1) // WIN
    ncols = nwin * WIN
    jt = npc_pad // P
    rj = ncols // P
    nfull = N_CORES * npc_pad
    nch = max(1, math.ceil(nfull / MAX_CHUNK))
    chunk_rows = math.ceil(nfull / nch)
    assert chunk_rows <= MAX_CHUNK

    src = np.concatenate([np.asarray(edge_index[0]), np.arange(N, dtype=np.int64)])
    dst = np.concatenate([np.asarray(edge_index[1]), np.arange(N, dtype=np.int64)])
    deg = np.bincount(dst, minlength=N).astype(np.float32)

    core = dst // npc
    dloc = dst - core * npc
    w = dloc // WIN
    s = (dloc - w * WIN) // SUB
    g_row = (src // npc) * npc_pad + (src % npc)
    k = g_row // chunk_rows

    # group id per edge: (core, w, s, k)
    n_gper = nwin * NSUB * nch
    gid = ((core * nwin + w) * NSUB + s) * nch + k
    counts = np.bincount(gid, minlength=N_CORES * n_gper)
    B = max(1, int(math.ceil(counts.max() / P)))

    # ---- block layout (identical across cores; data differs) ----
    # quads of WPC windows; call = (quad, k) covers len(quad)*NSUB*B blocks.
    quads = [list(range(q, min(q + WPC, nwin))) for q in range(0, nwin, WPC)]
    # block base for each (w, s, k)
    blk_base = np.zeros((nwin, NSUB, nch), np.int64)
    calls = []          # (k, windows, blk_start, nblk)
    nxt = 0
    for wq in quads:
        L = len(wq)
        for kk in range(nch):
            calls.append((kk, wq, nxt, L * NSUB * B))
            for wl, ww in enumerate(wq):
                for ss in range(NSUB):
                    blk_base[ww, ss, kk] = nxt + (wl * NSUB + ss) * B
            nxt += L * NSUB * B
    totblk = nxt
    slots = totblk * P

    # ---- slot assignment ----
    if SORT_SRC:
        order = np.lexsort((g_row, gid))
    else:
        order = np.argsort(gid, kind="stable")
    gid_s = gid[order]
    starts = np.zeros(N_CORES * n_gper + 1, np.int64)
    starts[1:] = np.cumsum(counts)
    pos = np.arange(len(order), dtype=np.int64) - starts[gid_s]
    assert pos.max() < B * P

    core_s = gid_s // n_gper
    rem = gid_s - core_s * n_gper
    w_s = rem // (NSUB * nch)
    rem2 = rem - w_s * (NSUB * nch)
    s_s = rem2 // nch
    k_s = rem2 - s_s * nch
    slot = blk_base[w_s, s_s, k_s] * P + pos

    if PAD_MODE == "spread":
        # pads must not hammer one HBM row: spread them over the chunk
        pad_fill = np.broadcast_to(
            ((np.arange(slots, dtype=np.int64) * 97) % chunk_rows).astype(np.int16),
            (N_CORES, slots))
        idx16 = pad_fill.copy()
    else:
        idx16 = np.zeros((N_CORES, slots), np.int16)
    dstrel = np.full((N_CORES, slots), -1.0, np.float32)
    idx16[core_s, slot] = (g_row[order] - k_s * chunk_rows).astype(np.int16)
    dstrel[core_s, slot] = (dloc[order] % SUB).astype(np.float32)
    if PAD_MODE == "ffill":
        # pads inherit the nearest preceding real index: the duplicate read
        # hits the same (already-open) HBM row, so pads are nearly free
        real = np.zeros((N_CORES, slots), bool)
        real[core_s, slot] = True
        pos = np.where(real, np.arange(slots)[None, :], 0)
        last = np.maximum.accumulate(pos, axis=1)
        idx16 = np.take_along_axis(idx16, last, axis=1)

    # device layouts
    idx_dev = np.stack([_wrap_idx16(idx16[c]) for c in range(N_CORES)])
    dst_dev = np.ascontiguousarray(
        dstrel.reshape(N_CORES, totblk, P).transpose(0, 2, 1)).astype(ml_dtypes.bfloat16)
    iota = np.ascontiguousarray(
        np.broadcast_to(np.arange(SUB, dtype=np.float32), (P, SUB))).astype(ml_dtypes.bfloat16)

    in_maps = []
    for c in range(N_CORES):
        deg_c = np.ones(npc_pad, np.float32)
        deg_c[:npc] = deg[c * npc:(c + 1) * npc]
        deg_wrap = np.ascontiguousarray(deg_c.reshape(jt, P).T)
        deg_rows_c = np.ones(ncols, np.float32)
        deg_rows_c[:npc] = deg[c * npc:(c + 1) * npc]
        x_c = np.zeros((npc_pad, P), np.float32)
        x_c[:npc] = x[c * npc:(c + 1) * npc]
        in_maps.append({
            "x": x_c,
            "w": np.ascontiguousarray(W, dtype=np.float32),
            "bias": np.asarray(b, np.float32).reshape(P, 1),
            "alpha": np.asarray(alpha, np.float32).reshape(P, 1),
            "deg_wrap": deg_wrap,
            "deg_rows": deg_rows_c.reshape(rj, P),
            "idx16": idx_dev[c],
            "dstrel": dst_dev[c],
            "iota": iota,
        })

    meta = dict(npc=npc, npc_pad=npc_pad, nwin=nwin, ncols=ncols, jt=jt, rj=rj,
                nch=nch, chunk_rows=chunk_rows, B=B, totblk=totblk,
                calls=calls, blk_base=blk_base, nfull=nfull)
    return in_maps, meta


def _build_program(meta):
    npc_pad = meta["npc_pad"]
    nwin = meta["nwin"]
    ncols = meta["ncols"]
    jt = meta["jt"]
    rj = meta["rj"]
    nch = meta["nch"]
    chunk_rows = meta["chunk_rows"]
    B = meta["B"]
    totblk = meta["totblk"]
    calls = meta["calls"]
    blk_base = meta["blk_base"]
    nfull = meta["nfull"]
    Act = mybir.ActivationFunctionType

    nc = bacc.Bacc("TRN2", target_bir_lowering=False, debug=False,
                   num_devices=N_CORES, num_swdge_queues=NQ)

    x_d = nc.dram_tensor("x", [npc_pad, P], F32, kind="ExternalInput").ap()
    w_d = nc.dram_tensor("w", [P, P], F32, kind="ExternalInput").ap()
    b_d = nc.dram_tensor("bias", [P, 1], F32, kind="ExternalInput").ap()
    alpha_d = nc.dram_tensor("alpha", [P, 1], F32, kind="ExternalInput").ap()
    degw_d = nc.dram_tensor("deg_wrap", [P, jt], F32, kind="ExternalInput").ap()
    degr_d = nc.dram_tensor("deg_rows", [rj, P], F32, kind="ExternalInput").ap()
    idx_d = nc.dram_tensor("idx16", [P, totblk * P // 16], I16,
                           kind="ExternalInput").ap()
    dstr_d = nc.dram_tensor("dstrel", [P, totblk], BF16, kind="ExternalInput").ap()
    iota_d = nc.dram_tensor("iota", [P, SUB], BF16, kind="ExternalInput").ap()
    out_d = nc.dram_tensor("out", [npc_pad, P], F32, kind="ExternalOutput").ap()

    _dbg0 = os.environ.get("GCN_DEBUG", "")
    with tile.TileContext(nc) as tc:
        with tile.ExitStack() as top:
            cpool = top.enter_context(tc.tile_pool(name="const", bufs=1))
            dpool = top.enter_context(tc.tile_pool(name="dram", bufs=1, space="DRAM"))

            # ---- constants ----
            w_f32 = cpool.tile([P, P], F32, name="w_f32")
            nc.sync.dma_start(out=w_f32[:], in_=w_d[:])
            w_bf = cpool.tile([P, P], BF16, name="w_bf")
            nc.vector.tensor_copy(out=w_bf[:], in_=w_f32[:])

            b_sb = cpool.tile([P, 1], F32, name="b_sb")
            nc.sync.dma_start(out=b_sb[:], in_=b_d[:])
            negb_sb = cpool.tile([P, 1], F32, name="negb_sb")
            nc.vector.tensor_scalar_mul(negb_sb[:], b_sb[:], -1.0)
            alpha_sb = cpool.tile([P, 1], F32, name="alpha_sb")
            nc.sync.dma_start(out=alpha_sb[:], in_=alpha_d[:])
            iota_sb = cpool.tile([P, SUB], BF16, name="iota_sb")
            nc.sync.dma_start(out=iota_sb[:], in_=iota_d[:])

            ident_bf = cpool.tile([P, P], BF16, name="ident_bf")
            make_identity(nc, ident_bf[:])
            ident_f32 = cpool.tile([P, P], F32, name="ident_f32")
            make_identity(nc, ident_f32[:])

            # dinv wrapped (per-partition scale for phase 1)
            degw_sb = cpool.tile([P, jt], F32, name="degw_sb")
            nc.sync.dma_start(out=degw_sb[:], in_=degw_d[:])
            dinvw_sb = cpool.tile([P, jt], F32, name="dinvw_sb")
            nc.vector.reciprocal(dinvw_sb[:], degw_sb[:])
            nc.scalar.sqrt(dinvw_sb[:], dinvw_sb[:])

            # dinv natural order -> DRAM (for free-dim broadcast loads)
            degr_sb = cpool.tile([rj, P], F32, name="degr_sb")
            nc.sync.dma_start(out=degr_sb[:], in_=degr_d[:])
            dinvr_sb = cpool.tile([rj, P], F32, name="dinvr_sb")
            nc.vector.reciprocal(dinvr_sb[:], degr_sb[:])
            nc.scalar.sqrt(dinvr_sb[:], dinvr_sb[:])
            dinv_dram = dpool.tile([ncols], F32, name="dinv_dram")
            nc.sync.dma_start(
                out=dinv_dram[:].rearrange("(r k) -> r k", r=rj), in_=dinvr_sb[:])

            # edge tables
            idx_sb = cpool.tile([P, totblk * P // 16], I16, name="idx_sb")
            nc.sync.dma_start(out=idx_sb[:], in_=idx_d[:])
            dstrel_sb = cpool.tile([P, totblk], BF16, name="dstrel_sb")
            nc.sync.dma_start(out=dstrel_sb[:], in_=dstr_d[:])

            g_c = dpool.tile([npc_pad, P], BF16, name="g_c")
            g_full = dpool.tile([nfull, P], BF16, addr_space="Shared", name="g_full")

            # ---- phase 1: g_c = dinv * (x @ W) ----
            if "noph1" not in _dbg0:
                with tile.ExitStack() as ph1:
                    psT_pool = ph1.enter_context(
                        tc.tile_pool(name="ph1psT", bufs=2, space="PSUM"))
                    psH_pool = ph1.enter_context(
                        tc.tile_pool(name="ph1psH", bufs=2, space="PSUM"))
                    ph1_pool = ph1.enter_context(tc.tile_pool(name="ph1sb", bufs=3))
                    x_all = ph1_pool.tile([P, jt, P], BF16, name="x_all", bufs=1)
                    nc.gpsimd.dma_start(
                        out=x_all[:], in_=x_d[:].rearrange("(j p) f -> p j f", p=P))
                    for t in range(jt):
                        xT_ps = psT_pool.tile([P, P], BF16, tag="xT", name="xT_ps")
                        nc.tensor.transpose(out=xT_ps[:], in_=x_all[:, t, :],
                                            identity=ident_bf[:])
                        xT_sb = ph1_pool.tile([P, P], BF16, tag="xTs", name="xT_sb")
                        nc.scalar.activation(out=xT_sb[:], in_=xT_ps[:], func=Act.Copy)
                        h_ps = psH_pool.tile([P, P], F32, tag="h", name="h_ps")
                        nc.tensor.matmul(out=h_ps[:], lhsT=xT_sb[:], rhs=w_bf[:],
                                         start=True, stop=True)
                        g_sb = ph1_pool.tile([P, P], BF16, tag="g", name="g_sb")
                        nc.scalar.activation(out=g_sb[:], in_=h_ps[:], func=Act.Copy,
                                             scale=dinvw_sb[:, t:t + 1])
                        nc.sync.dma_start(out=g_c[t * P:(t + 1) * P, :], in_=g_sb[:])

            # ---- phase 2: replicate g ----
            if "noag" not in os.environ.get("GCN_DEBUG", ""):
                nc.gpsimd.collective_compute(
                    "AllGather",
                    mybir.AluOpType.bypass,
                    replica_groups=[list(range(N_CORES))],
                    ins=[g_c[:].opt()],
                    outs=[g_full[:].opt()],
                )

            # ---- phase 3 ----
            _dbg = os.environ.get("GCN_DEBUG", "")
            if "noph3" in _dbg:
                calls = []
            with tile.ExitStack() as ph3:
                gat_pools = [ph3.enter_context(tc.tile_pool(name=f"gat{q}", bufs=2))
                             for q in range(NQ)]
                oh_pool = ph3.enter_context(tc.tile_pool(name="oh", bufs=2))
                psw_pool = ph3.enter_context(
                    tc.tile_pool(name="psw", bufs=6, space="PSUM"))
                ztr_pool = ph3.enter_context(
                    tc.tile_pool(name="ztr", bufs=2, space="PSUM"))
                epi_pool = ph3.enter_context(tc.tile_pool(name="epi", bufs=2))
                dv_pool = ph3.enter_context(tc.tile_pool(name="dv", bufs=2))

                psw = {}           # window -> psum tile
                done_k = {}        # window -> #chunks matmul'd
                for ci, (kk, wq, blk_start, nblk) in enumerate(calls):
                    q = ci % NQ
                    gat = gat_pools[q].tile([P, nblk, P], BF16, tag="gat",
                                            name="gat")
                    if "nogather" in _dbg:
                        nc.vector.memset(gat[:], 0.0)
                    else:
                        nc.gpsimd.dma_gather(
                            out_ap=gat[:],
                            in_ap=g_full[kk * chunk_rows:
                                         min((kk + 1) * chunk_rows, nfull), :],
                            idxs_ap=idx_sb[:, blk_start * P // 16:
                                           (blk_start + nblk) * P // 16],
                            num_idxs=nblk * P,
                            num_idxs_reg=nblk * P,
                            elem_size=P,
                            single_packet=SINGLE_PACKET,
                            queue_num=q,
                        )
                    oh = oh_pool.tile([P, nblk, SUB], BF16, tag="oh", name="oh")
                    if "nooh" in _dbg:
                        nc.vector.memset(oh[:], 0.0)
                    else:
                        nc.vector.tensor_tensor(
                            out=oh[:],
                            in0=dstrel_sb[:, blk_start:blk_start + nblk]
                                .unsqueeze(2).to_broadcast([P, nblk, SUB]),
                            in1=iota_sb[:].unsqueeze(1).to_broadcast([P, nblk, SUB]),
                            op=mybir.AluOpType.is_equal,
                        )
                    for ww in wq:
                        if ww not in psw:
                            psw[ww] = psw_pool.tile([P, WIN], F32, tag="psw",
                                                    name="psw")
                            nc.vector.memset(psw[ww][:], 0.0)
                            done_k[ww] = 0
                        if "nomm" not in _dbg:
                            for ss in range(NSUB):
                                for i in range(B):
                                    blk = blk_base[ww, ss, kk] - blk_start + i
                                    nc.tensor.matmul(
                                        out=psw[ww][:, ss * SUB:(ss + 1) * SUB],
                                        lhsT=gat[:, blk, :],
                                        rhs=oh[:, blk, :],
                                        start=False,
                                        stop=(kk == nch - 1 and i == B - 1),
                                        skip_group_check=True,
                                    )
                        done_k[ww] += 1
                        if done_k[ww] < nch:
                            continue
                        # ---- epilogue for window ww ----
                        pswt = psw.pop(ww)
                        dinv_win = dv_pool.tile([P, WIN], F32, tag="dv",
                                                name="dinv_win")
                        nc.sync.dma_start(
                            out=dinv_win[:],
                            in_=dinv_dram[ww * WIN:(ww + 1) * WIN]
                                .unsqueeze(0).partition_broadcast(P))
                        u = epi_pool.tile([P, WIN], F32, tag="u", name="u")
                        nc.vector.tensor_mul(u[:], pswt[:], dinv_win[:])
                        r = epi_pool.tile([P, WIN], F32, tag="r", name="r")
                        nc.scalar.activation(out=r[:], in_=u[:], func=Act.Relu,
                                             bias=b_sb[:, 0:1], scale=1.0)
                        q = epi_pool.tile([P, WIN], F32, tag="q", name="q")
                        nc.scalar.activation(out=q[:], in_=u[:], func=Act.Relu,
                                             bias=negb_sb[:, 0:1], scale=-1.0)
                        tq = epi_pool.tile([P, WIN], F32, tag="tq", name="tq")
                        nc.scalar.activation(out=tq[:], in_=q[:], func=Act.Copy,
                                             scale=alpha_sb[:, 0:1])
                        z = epi_pool.tile([P, WIN], F32, tag="z", name="z")
                        nc.vector.tensor_sub(z[:], r[:], tq[:])
                        # transpose back and write natural rows
                        for j in range(NSUB):
                            row0 = ww * WIN + j * SUB
                            if row0 >= npc_pad:
                                break
                            zt_ps = ztr_pool.tile([P, P], F32, tag="zt",
                                                  name="zt_ps")
                            nc.tensor.transpose(out=zt_ps[:],
                                                in_=z[:, j * SUB:(j + 1) * SUB],
                                                identity=ident_f32[:])
                            zrow = epi_pool.tile([P, P], F32, tag="zrow",
                                                 name="zrow")
                            nc.scalar.activation(out=zrow[:], in_=zt_ps[:],
                                                 func=Act.Copy)
                            nrows = min(P, npc_pad - row0)
                            nc.sync.dma_start(out=out_d[row0:row0 + nrows, :],
                                              in_=zrow[:nrows, :])

    nc.compile()
    return nc


_CACHE = {}


def kernel(x, edge_index, W, b, alpha):
    x = np.asarray(x)
    edge_index = np.asarray(edge_index)

    in_maps, meta = _host_prep(x, edge_index, np.asarray(W), np.asarray(b),
                               np.asarray(alpha))
    key = (x.shape, edge_index.shape, meta["B"], meta["totblk"])
    if key not in _CACHE:
        _CACHE[key] = _build_program(meta)
    nc = _CACHE[key]

    r = run_bass_kernel_spmd(nc, in_maps, list(range(N_CORES)))
    npc = meta["npc"]
    out = np.concatenate([np.asarray(r.results[c]["out"])[:npc]
                          for c in range(N_CORES)], axis=0)
    return out.astype(np.float32)



# revision 14
# speedup vs baseline: 3.9834x; 3.9834x over previous
"""GCN encoder (GCNConv + PReLU) distributed Bass kernel for 8 TRN2 NeuronCores.

Reference computation:
    src/dst = edge_index with self loops appended
    deg[v]  = #edges with dst==v (incl. self loop)
    dinv    = rsqrt(deg)
    h       = x @ W
    agg[v]  = sum_{e: dst=v} dinv[src_e]*dinv[v]*h[src_e] + b
    out     = prelu(agg, alpha)

Factored form used on device:
    g[u]    = dinv[u] * h[u]
    out[v]  = prelu(dinv[v] * sum_{e: dst=v} g[src_e] + b)   (self loop = an edge)

Distribution: dst-node ownership sharded over 8 cores. Each core computes its
g shard (x @ W via PE transpose + matmul, scaled by rsqrt(deg)), an AllGather
replicates g (bf16), then each core gathers the 256B g rows of its edges with
dma_gather and segment-sums them on the TensorEngine: for each 128-edge block,
matmul(lhsT=g_rows[128e,128f], rhs=onehot[128e,128dst]) accumulates into a
PSUM [feat, 512dst] window. onehot = is_equal(dstrel, iota) built on VectorE.
Epilogue applies dinv[dst] (free-dim broadcast), bias and PReLU, transposes
back via PE, and writes natural-layout rows.

Host side does only integer index work: self loops, bincount, edge binning
into (window, subwindow, table-chunk) groups padded to 128-edge blocks, and
int16 gather-index tables (dma_gather needs idx<32768, so g is addressed in
<=32767-row chunks; edges are grouped by chunk).
"""

import math
import os

import numpy as np
import ml_dtypes

import concourse.bass as bass
import concourse.tile as tile
import concourse.bacc as bacc
from concourse import mybir
from concourse.bass_utils import run_bass_kernel_spmd
from concourse.masks import make_identity

N_CORES = 8
P = 128          # partitions / feature dim
WIN = 512        # dst columns per PSUM window (one f32 bank)
SUB = 128        # dst columns per one-hot subwindow
NSUB = WIN // SUB
WPC = int(os.environ.get("GCN_WPC", "1"))  # windows per dma_gather call group
NQ = int(os.environ.get("GCN_NQ", "4"))    # SWDGE queues (gather desc-gen core pairs)
MAX_CHUNK = int(os.environ.get("GCN_MAXCHUNK", "32767"))  # dma_gather int16 idx limit
SORT_SRC = os.environ.get("GCN_SORT", "0") == "1"  # sort gathers by source within group
PAD_MODE = os.environ.get("GCN_PAD", "spread")     # pad idx: "zero" | "spread" | "ffill"
VAR_B = os.environ.get("GCN_VARB", "1") == "1"     # per-group block counts (fewer pads)
SINGLE_PACKET = os.environ.get("GCN_SP", "0") == "1"

BF16 = mybir.dt.bfloat16
F32 = mybir.dt.float32
I16 = mybir.dt.int16


def _wrap_idx16(flat):
    """[n] int -> [128, n//16] int16: idx i at partition i%16 position i//16,
    replicated to all 8 Q7 core groups."""
    n = flat.shape[0]
    a = flat.reshape(n // 16, 16).T.astype(np.int16)      # [16, n//16]
    return np.tile(a, (8, 1))                              # [128, n//16]


def _host_prep(x, edge_index, W, b, alpha):
    N, D = x.shape
    assert D == P and N % N_CORES == 0
    npc = N // N_CORES
    npc_pad = ((npc + P - 1) // P) * P
    nwin = (npc + WIN - 1) // WIN
    ncols = nwin * WIN
    jt = npc_pad // P
    rj = ncols // P
    nfull = N_CORES * npc_pad
    nch = max(1, math.ceil(nfull / MAX_CHUNK))
    chunk_rows = math.ceil(nfull / nch)
    assert chunk_rows <= MAX_CHUNK

    src = np.concatenate([np.asarray(edge_index[0]), np.arange(N, dtype=np.int64)])
    dst = np.concatenate([np.asarray(edge_index[1]), np.arange(N, dtype=np.int64)])
    deg = np.bincount(dst, minlength=N).astype(np.float32)

    core = dst // npc
    dloc = dst - core * npc
    w = dloc // WIN
    s = (dloc - w * WIN) // SUB
    g_row = (src // npc) * npc_pad + (src % npc)
    k = g_row // chunk_rows

    # group id per edge: (core, w, s, k)
    n_gper = nwin * NSUB * nch
    gid = ((core * nwin + w) * NSUB + s) * nch + k
    counts = np.bincount(gid, minlength=N_CORES * n_gper)
    B = max(1, int(math.ceil(counts.max() / P)))

    # per-(w,s,k) block count: max over the 8 cores only (the program is SPMD
    # so the block layout must be identical across cores, but it can vary by
    # group) -- saves ~17% of gather descriptors vs one global max
    if VAR_B:
        gmax = counts.reshape(N_CORES, nwin, NSUB, nch).max(axis=0)
        Bg = np.maximum(1, -(-gmax // P)).astype(np.int64)      # [nwin, NSUB, nch]
    else:
        Bg = np.full((nwin, NSUB, nch), B, np.int64)

    # ---- block layout (identical across cores; data differs) ----
    # quads of WPC windows; call = (quad, k) covers sum of the quad's Bg blocks.
    quads = [list(range(q, min(q + WPC, nwin))) for q in range(0, nwin, WPC)]
    # block base for each (w, s, k)
    blk_base = np.zeros((nwin, NSUB, nch), np.int64)
    calls = []          # (k, windows, blk_start, nblk)
    nxt = 0
    for wq in quads:
        for kk in range(nch):
            start = nxt
            for ww in wq:
                for ss in range(NSUB):
                    blk_base[ww, ss, kk] = nxt
                    nxt += Bg[ww, ss, kk]
            calls.append((kk, wq, start, nxt - start))
    totblk = nxt
    slots = totblk * P

    # ---- slot assignment ----
    if SORT_SRC:
        order = np.lexsort((g_row, gid))
    else:
        order = np.argsort(gid, kind="stable")
    gid_s = gid[order]
    starts = np.zeros(N_CORES * n_gper + 1, np.int64)
    starts[1:] = np.cumsum(counts)
    pos = np.arange(len(order), dtype=np.int64) - starts[gid_s]

    core_s = gid_s // n_gper
    rem = gid_s - core_s * n_gper
    w_s = rem // (NSUB * nch)
    rem2 = rem - w_s * (NSUB * nch)
    s_s = rem2 // nch
    k_s = rem2 - s_s * nch
    assert (pos < Bg[w_s, s_s, k_s] * P).all()
    slot = blk_base[w_s, s_s, k_s] * P + pos

    if PAD_MODE == "spread":
        # pads must not hammer one HBM row: spread them over the chunk
        pad_fill = np.broadcast_to(
            ((np.arange(slots, dtype=np.int64) * 97) % chunk_rows).astype(np.int16),
            (N_CORES, slots))
        idx16 = pad_fill.copy()
    else:
        idx16 = np.zeros((N_CORES, slots), np.int16)
    dstrel = np.full((N_CORES, slots), -1.0, np.float32)
    idx16[core_s, slot] = (g_row[order] - k_s * chunk_rows).astype(np.int16)
    dstrel[core_s, slot] = (dloc[order] % SUB).astype(np.float32)
    if PAD_MODE == "ffill":
        # pads inherit the nearest preceding real index: the duplicate read
        # hits the same (already-open) HBM row, so pads are nearly free
        real = np.zeros((N_CORES, slots), bool)
        real[core_s, slot] = True
        pos = np.where(real, np.arange(slots)[None, :], 0)
        last = np.maximum.accumulate(pos, axis=1)
        idx16 = np.take_along_axis(idx16, last, axis=1)

    # device layouts
    idx_dev = np.stack([_wrap_idx16(idx16[c]) for c in range(N_CORES)])
    dst_dev = np.ascontiguousarray(
        dstrel.reshape(N_CORES, totblk, P).transpose(0, 2, 1)).astype(ml_dtypes.bfloat16)
    iota = np.ascontiguousarray(
        np.broadcast_to(np.arange(SUB, dtype=np.float32), (P, SUB))).astype(ml_dtypes.bfloat16)

    in_maps = []
    for c in range(N_CORES):
        deg_c = np.ones(npc_pad, np.float32)
        deg_c[:npc] = deg[c * npc:(c + 1) * npc]
        deg_wrap = np.ascontiguousarray(deg_c.reshape(jt, P).T)
        deg_rows_c = np.ones(ncols, np.float32)
        deg_rows_c[:npc] = deg[c * npc:(c + 1) * npc]
        x_c = np.zeros((npc_pad, P), np.float32)
        x_c[:npc] = x[c * npc:(c + 1) * npc]
        in_maps.append({
            "x": x_c,
            "w": np.ascontiguousarray(W, dtype=np.float32),
            "bias": np.asarray(b, np.float32).reshape(P, 1),
            "alpha": np.asarray(alpha, np.float32).reshape(P, 1),
            "deg_wrap": deg_wrap,
            "deg_rows": deg_rows_c.reshape(rj, P),
            "idx16": idx_dev[c],
            "dstrel": dst_dev[c],
            "iota": iota,
        })

    meta = dict(npc=npc, npc_pad=npc_pad, nwin=nwin, ncols=ncols, jt=jt, rj=rj,
                nch=nch, chunk_rows=chunk_rows, B=B, Bg=Bg, totblk=totblk,
                nblk_max=max(c[3] for c in calls),
                calls=calls, blk_base=blk_base, nfull=nfull)
    return in_maps, meta


def _build_program(meta):
    npc_pad = meta["npc_pad"]
    nwin = meta["nwin"]
    ncols = meta["ncols"]
    jt = meta["jt"]
    rj = meta["rj"]
    nch = meta["nch"]
    chunk_rows = meta["chunk_rows"]
    Bg = meta["Bg"]
    nblk_max = meta["nblk_max"]
    totblk = meta["totblk"]
    calls = meta["calls"]
    blk_base = meta["blk_base"]
    nfull = meta["nfull"]
    Act = mybir.ActivationFunctionType

    nc = bacc.Bacc("TRN2", target_bir_lowering=False, debug=False,
                   num_devices=N_CORES, num_swdge_queues=NQ)

    x_d = nc.dram_tensor("x", [npc_pad, P], F32, kind="ExternalInput").ap()
    w_d = nc.dram_tensor("w", [P, P], F32, kind="ExternalInput").ap()
    b_d = nc.dram_tensor("bias", [P, 1], F32, kind="ExternalInput").ap()
    alpha_d = nc.dram_tensor("alpha", [P, 1], F32, kind="ExternalInput").ap()
    degw_d = nc.dram_tensor("deg_wrap", [P, jt], F32, kind="ExternalInput").ap()
    degr_d = nc.dram_tensor("deg_rows", [rj, P], F32, kind="ExternalInput").ap()
    idx_d = nc.dram_tensor("idx16", [P, totblk * P // 16], I16,
                           kind="ExternalInput").ap()
    dstr_d = nc.dram_tensor("dstrel", [P, totblk], BF16, kind="ExternalInput").ap()
    iota_d = nc.dram_tensor("iota", [P, SUB], BF16, kind="ExternalInput").ap()
    out_d = nc.dram_tensor("out", [npc_pad, P], F32, kind="ExternalOutput").ap()

    _dbg0 = os.environ.get("GCN_DEBUG", "")
    with tile.TileContext(nc) as tc:
        with tile.ExitStack() as top:
            cpool = top.enter_context(tc.tile_pool(name="const", bufs=1))
            dpool = top.enter_context(tc.tile_pool(name="dram", bufs=1, space="DRAM"))

            # ---- constants ----
            w_f32 = cpool.tile([P, P], F32, name="w_f32")
            nc.sync.dma_start(out=w_f32[:], in_=w_d[:])
            w_bf = cpool.tile([P, P], BF16, name="w_bf")
            nc.vector.tensor_copy(out=w_bf[:], in_=w_f32[:])

            b_sb = cpool.tile([P, 1], F32, name="b_sb")
            nc.sync.dma_start(out=b_sb[:], in_=b_d[:])
            negb_sb = cpool.tile([P, 1], F32, name="negb_sb")
            nc.vector.tensor_scalar_mul(negb_sb[:], b_sb[:], -1.0)
            alpha_sb = cpool.tile([P, 1], F32, name="alpha_sb")
            nc.sync.dma_start(out=alpha_sb[:], in_=alpha_d[:])
            iota_sb = cpool.tile([P, SUB], BF16, name="iota_sb")
            nc.sync.dma_start(out=iota_sb[:], in_=iota_d[:])

            ident_bf = cpool.tile([P, P], BF16, name="ident_bf")
            make_identity(nc, ident_bf[:])
            ident_f32 = cpool.tile([P, P], F32, name="ident_f32")
            make_identity(nc, ident_f32[:])

            # dinv wrapped (per-partition scale for phase 1)
            degw_sb = cpool.tile([P, jt], F32, name="degw_sb")
            nc.sync.dma_start(out=degw_sb[:], in_=degw_d[:])
            dinvw_sb = cpool.tile([P, jt], F32, name="dinvw_sb")
            nc.vector.reciprocal(dinvw_sb[:], degw_sb[:])
            nc.scalar.sqrt(dinvw_sb[:], dinvw_sb[:])

            # dinv natural order -> DRAM (for free-dim broadcast loads)
            degr_sb = cpool.tile([rj, P], F32, name="degr_sb")
            nc.sync.dma_start(out=degr_sb[:], in_=degr_d[:])
            dinvr_sb = cpool.tile([rj, P], F32, name="dinvr_sb")
            nc.vector.reciprocal(dinvr_sb[:], degr_sb[:])
            nc.scalar.sqrt(dinvr_sb[:], dinvr_sb[:])
            dinv_dram = dpool.tile([ncols], F32, name="dinv_dram")
            nc.sync.dma_start(
                out=dinv_dram[:].rearrange("(r k) -> r k", r=rj), in_=dinvr_sb[:])

            # edge tables
            idx_sb = cpool.tile([P, totblk * P // 16], I16, name="idx_sb")
            nc.sync.dma_start(out=idx_sb[:], in_=idx_d[:])
            dstrel_sb = cpool.tile([P, totblk], BF16, name="dstrel_sb")
            nc.sync.dma_start(out=dstrel_sb[:], in_=dstr_d[:])

            g_c = dpool.tile([npc_pad, P], BF16, name="g_c")
            g_full = dpool.tile([nfull, P], BF16, addr_space="Shared", name="g_full")

            # ---- phase 1: g_c = dinv * (x @ W) ----
            if "noph1" not in _dbg0:
                with tile.ExitStack() as ph1:
                    psT_pool = ph1.enter_context(
                        tc.tile_pool(name="ph1psT", bufs=2, space="PSUM"))
                    psH_pool = ph1.enter_context(
                        tc.tile_pool(name="ph1psH", bufs=2, space="PSUM"))
                    ph1_pool = ph1.enter_context(tc.tile_pool(name="ph1sb", bufs=3))
                    x_all = ph1_pool.tile([P, jt, P], BF16, name="x_all", bufs=1)
                    nc.gpsimd.dma_start(
                        out=x_all[:], in_=x_d[:].rearrange("(j p) f -> p j f", p=P))
                    for t in range(jt):
                        xT_ps = psT_pool.tile([P, P], BF16, tag="xT", name="xT_ps")
                        nc.tensor.transpose(out=xT_ps[:], in_=x_all[:, t, :],
                                            identity=ident_bf[:])
                        xT_sb = ph1_pool.tile([P, P], BF16, tag="xTs", name="xT_sb")
                        nc.scalar.activation(out=xT_sb[:], in_=xT_ps[:], func=Act.Copy)
                        h_ps = psH_pool.tile([P, P], F32, tag="h", name="h_ps")
                        nc.tensor.matmul(out=h_ps[:], lhsT=xT_sb[:], rhs=w_bf[:],
                                         start=True, stop=True)
                        g_sb = ph1_pool.tile([P, P], BF16, tag="g", name="g_sb")
                        nc.scalar.activation(out=g_sb[:], in_=h_ps[:], func=Act.Copy,
                                             scale=dinvw_sb[:, t:t + 1])
                        nc.sync.dma_start(out=g_c[t * P:(t + 1) * P, :], in_=g_sb[:])

            # ---- phase 2: replicate g ----
            if "noag" not in os.environ.get("GCN_DEBUG", ""):
                nc.gpsimd.collective_compute(
                    "AllGather",
                    mybir.AluOpType.bypass,
                    replica_groups=[list(range(N_CORES))],
                    ins=[g_c[:].opt()],
                    outs=[g_full[:].opt()],
                )

            # ---- phase 3 ----
            _dbg = os.environ.get("GCN_DEBUG", "")
            if "noph3" in _dbg:
                calls = []
            with tile.ExitStack() as ph3:
                gat_pools = [ph3.enter_context(tc.tile_pool(name=f"gat{q}", bufs=2))
                             for q in range(NQ)]
                oh_pool = ph3.enter_context(tc.tile_pool(name="oh", bufs=2))
                psw_pool = ph3.enter_context(
                    tc.tile_pool(name="psw", bufs=6, space="PSUM"))
                ztr_pool = ph3.enter_context(
                    tc.tile_pool(name="ztr", bufs=2, space="PSUM"))
                epi_pool = ph3.enter_context(tc.tile_pool(name="epi", bufs=2))
                dv_pool = ph3.enter_context(tc.tile_pool(name="dv", bufs=2))

                psw = {}           # window -> psum tile
                done_k = {}        # window -> #chunks matmul'd
                for ci, (kk, wq, blk_start, nblk) in enumerate(calls):
                    q = ci % NQ
                    gat = gat_pools[q].tile([P, nblk_max, P], BF16, tag="gat",
                                            name="gat")
                    if "nogather" in _dbg:
                        nc.vector.memset(gat[:, :nblk, :], 0.0)
                    else:
                        nc.gpsimd.dma_gather(
                            out_ap=gat[:, :nblk, :],
                            in_ap=g_full[kk * chunk_rows:
                                         min((kk + 1) * chunk_rows, nfull), :],
                            idxs_ap=idx_sb[:, blk_start * P // 16:
                                           (blk_start + nblk) * P // 16],
                            num_idxs=nblk * P,
                            num_idxs_reg=nblk * P,
                            elem_size=P,
                            single_packet=SINGLE_PACKET,
                            queue_num=q,
                        )
                    oh = oh_pool.tile([P, nblk_max, SUB], BF16, tag="oh",
                                      name="oh")
                    if "nooh" in _dbg:
                        nc.vector.memset(oh[:, :nblk, :], 0.0)
                    else:
                        nc.vector.tensor_tensor(
                            out=oh[:, :nblk, :],
                            in0=dstrel_sb[:, blk_start:blk_start + nblk]
                                .unsqueeze(2).to_broadcast([P, nblk, SUB]),
                            in1=iota_sb[:].unsqueeze(1).to_broadcast([P, nblk, SUB]),
                            op=mybir.AluOpType.is_equal,
                        )
                    for ww in wq:
                        if ww not in psw:
                            psw[ww] = psw_pool.tile([P, WIN], F32, tag="psw",
                                                    name="psw")
                            nc.vector.memset(psw[ww][:], 0.0)
                            done_k[ww] = 0
                        if "nomm" not in _dbg:
                            for ss in range(NSUB):
                                nb = int(Bg[ww, ss, kk])
                                for i in range(nb):
                                    blk = blk_base[ww, ss, kk] - blk_start + i
                                    nc.tensor.matmul(
                                        out=psw[ww][:, ss * SUB:(ss + 1) * SUB],
                                        lhsT=gat[:, blk, :],
                                        rhs=oh[:, blk, :],
                                        start=False,
                                        stop=(kk == nch - 1 and i == nb - 1),
                                        skip_group_check=True,
                                    )
                        done_k[ww] += 1
                        if done_k[ww] < nch:
                            continue
                        # ---- epilogue for window ww ----
                        pswt = psw.pop(ww)
                        dinv_win = dv_pool.tile([P, WIN], F32, tag="dv",
                                                name="dinv_win")
                        nc.sync.dma_start(
                            out=dinv_win[:],
                            in_=dinv_dram[ww * WIN:(ww + 1) * WIN]
                                .unsqueeze(0).partition_broadcast(P))
                        u = epi_pool.tile([P, WIN], F32, tag="u", name="u")
                        nc.vector.tensor_mul(u[:], pswt[:], dinv_win[:])
                        r = epi_pool.tile([P, WIN], F32, tag="r", name="r")
                        nc.scalar.activation(out=r[:], in_=u[:], func=Act.Relu,
                                             bias=b_sb[:, 0:1], scale=1.0)
                        q = epi_pool.tile([P, WIN], F32, tag="q", name="q")
                        nc.scalar.activation(out=q[:], in_=u[:], func=Act.Relu,
                                             bias=negb_sb[:, 0:1], scale=-1.0)
                        tq = epi_pool.tile([P, WIN], F32, tag="tq", name="tq")
                        nc.scalar.activation(out=tq[:], in_=q[:], func=Act.Copy,
                                             scale=alpha_sb[:, 0:1])
                        z = epi_pool.tile([P, WIN], F32, tag="z", name="z")
                        nc.vector.tensor_sub(z[:], r[:], tq[:])
                        # transpose back and write natural rows
                        for j in range(NSUB):
                            row0 = ww * WIN + j * SUB
                            if row0 >= npc_pad:
                                break
                            zt_ps = ztr_pool.tile([P, P], F32, tag="zt",
                                                  name="zt_ps")
                            nc.tensor.transpose(out=zt_ps[:],
                                                in_=z[:, j * SUB:(j + 1) * SUB],
                                                identity=ident_f32[:])
                            zrow = epi_pool.tile([P, P], F32, tag="zrow",
                                                 name="zrow")
                            nc.scalar.activation(out=zrow[:], in_=zt_ps[:],
                                                 func=Act.Copy)
                            nrows = min(P, npc_pad - row0)
                            nc.sync.dma_start(out=out_d[row0:row0 + nrows, :],
                                              in_=zrow[:nrows, :])

    nc.compile()
    return nc


_CACHE = {}


def kernel(x, edge_index, W, b, alpha):
    x = np.asarray(x)
    edge_index = np.asarray(edge_index)

    in_maps, meta = _host_prep(x, edge_index, np.asarray(W), np.asarray(b),
                               np.asarray(alpha))
    key = (x.shape, edge_index.shape, meta["B"], meta["totblk"])
    if key not in _CACHE:
        _CACHE[key] = _build_program(meta)
    nc = _CACHE[key]

    r = run_bass_kernel_spmd(nc, in_maps, list(range(N_CORES)))
    npc = meta["npc"]
    out = np.concatenate([np.asarray(r.results[c]["out"])[:npc]
                          for c in range(N_CORES)], axis=0)
    return out.astype(np.float32)



# revision 15
# speedup vs baseline: 4.1842x; 1.0504x over previous
"""GCN encoder (GCNConv + PReLU) distributed Bass kernel for 8 TRN2 NeuronCores.

Reference computation:
    src/dst = edge_index with self loops appended
    deg[v]  = #edges with dst==v (incl. self loop)
    dinv    = rsqrt(deg)
    h       = x @ W
    agg[v]  = sum_{e: dst=v} dinv[src_e]*dinv[v]*h[src_e] + b
    out     = prelu(agg, alpha)

Factored form used on device:
    g[u]    = dinv[u] * h[u]
    out[v]  = prelu(dinv[v] * sum_{e: dst=v} g[src_e] + b)   (self loop = an edge)

Distribution: dst-node ownership sharded over 8 cores. Each core computes its
g shard (x @ W via PE transpose + matmul, scaled by rsqrt(deg)), an AllGather
replicates g (bf16), then each core gathers the 256B g rows of its edges with
dma_gather and segment-sums them on the TensorEngine: for each 128-edge block,
matmul(lhsT=g_rows[128e,128f], rhs=onehot[128e,128dst]) accumulates into a
PSUM [feat, 512dst] window. onehot = is_equal(dstrel, iota) built on VectorE.
Epilogue applies dinv[dst] (free-dim broadcast), bias and PReLU, transposes
back via PE, and writes natural-layout rows.

Host side does only integer index work: self loops, bincount, edge binning
into (window, subwindow, table-chunk) groups padded to 128-edge blocks, and
int16 gather-index tables (dma_gather needs idx<32768, so g is addressed in
<=32767-row chunks; edges are grouped by chunk).
"""

import math
import os

import numpy as np
import ml_dtypes

import concourse.bass as bass
import concourse.tile as tile
import concourse.bacc as bacc
from concourse import mybir
from concourse.bass_utils import run_bass_kernel_spmd
from concourse.masks import make_identity

N_CORES = 8
P = 128          # partitions / feature dim
WIN = 512        # dst columns per PSUM window (one f32 bank)
SUB = 128        # dst columns per one-hot subwindow
NSUB = WIN // SUB
WPC = int(os.environ.get("GCN_WPC", "1"))  # windows per dma_gather call group
NQ = int(os.environ.get("GCN_NQ", "4"))    # SWDGE queues (gather desc-gen core pairs)
MAX_CHUNK = int(os.environ.get("GCN_MAXCHUNK", "32767"))  # dma_gather int16 idx limit
SORT_SRC = os.environ.get("GCN_SORT", "0") == "1"  # sort gathers by source within group
PAD_MODE = os.environ.get("GCN_PAD", "spread")     # pad idx: "zero" | "spread" | "ffill"
VAR_B = os.environ.get("GCN_VARB", "1") == "1"     # per-group block counts (fewer pads)
SINGLE_PACKET = os.environ.get("GCN_SP", "0") == "1"

BF16 = mybir.dt.bfloat16
F32 = mybir.dt.float32
I16 = mybir.dt.int16


def _wrap_idx16(flat):
    """[n] int -> [128, n//16] int16: idx i at partition i%16 position i//16,
    replicated to all 8 Q7 core groups."""
    n = flat.shape[0]
    a = flat.reshape(n // 16, 16).T.astype(np.int16)      # [16, n//16]
    return np.tile(a, (8, 1))                              # [128, n//16]


def _host_prep(x, edge_index, W, b, alpha):
    N, D = x.shape
    assert D == P and N % N_CORES == 0
    npc = N // N_CORES
    npc_pad = ((npc + P - 1) // P) * P
    nwin = (npc + WIN - 1) // WIN
    ncols = nwin * WIN
    jt = npc_pad // P
    rj = ncols // P
    nfull = N_CORES * npc_pad
    nch = max(1, math.ceil(nfull / MAX_CHUNK))
    chunk_rows = math.ceil(nfull / nch)
    assert chunk_rows <= MAX_CHUNK

    src = np.concatenate([np.asarray(edge_index[0]), np.arange(N, dtype=np.int64)])
    dst = np.concatenate([np.asarray(edge_index[1]), np.arange(N, dtype=np.int64)])
    deg = np.bincount(dst, minlength=N).astype(np.float32)

    core = dst // npc
    dloc = dst - core * npc
    w = dloc // WIN
    s = (dloc - w * WIN) // SUB
    g_row = (src // npc) * npc_pad + (src % npc)
    k = g_row // chunk_rows

    # group id per edge: (core, w, s, k)
    n_gper = nwin * NSUB * nch
    gid = ((core * nwin + w) * NSUB + s) * nch + k
    counts = np.bincount(gid, minlength=N_CORES * n_gper)
    B = max(1, int(math.ceil(counts.max() / P)))

    # per-(w,s,k) block count: max over the 8 cores only (the program is SPMD
    # so the block layout must be identical across cores, but it can vary by
    # group) -- saves ~17% of gather descriptors vs one global max
    if VAR_B:
        gmax = counts.reshape(N_CORES, nwin, NSUB, nch).max(axis=0)
        Bg = np.maximum(1, -(-gmax // P)).astype(np.int64)      # [nwin, NSUB, nch]
    else:
        Bg = np.full((nwin, NSUB, nch), B, np.int64)

    # ---- block layout (identical across cores; data differs) ----
    # quads of WPC windows; call = (quad, k) covers sum of the quad's Bg blocks.
    quads = [list(range(q, min(q + WPC, nwin))) for q in range(0, nwin, WPC)]
    # block base for each (w, s, k)
    blk_base = np.zeros((nwin, NSUB, nch), np.int64)
    calls = []          # (k, windows, blk_start, nblk)
    nxt = 0
    for wq in quads:
        for kk in range(nch):
            start = nxt
            for ww in wq:
                for ss in range(NSUB):
                    blk_base[ww, ss, kk] = nxt
                    nxt += Bg[ww, ss, kk]
            calls.append((kk, wq, start, nxt - start))
    totblk = nxt
    slots = totblk * P

    # ---- slot assignment ----
    if SORT_SRC:
        order = np.lexsort((g_row, gid))
    else:
        order = np.argsort(gid, kind="stable")
    gid_s = gid[order]
    starts = np.zeros(N_CORES * n_gper + 1, np.int64)
    starts[1:] = np.cumsum(counts)
    pos = np.arange(len(order), dtype=np.int64) - starts[gid_s]

    core_s = gid_s // n_gper
    rem = gid_s - core_s * n_gper
    w_s = rem // (NSUB * nch)
    rem2 = rem - w_s * (NSUB * nch)
    s_s = rem2 // nch
    k_s = rem2 - s_s * nch
    assert (pos < Bg[w_s, s_s, k_s] * P).all()
    slot = blk_base[w_s, s_s, k_s] * P + pos

    if PAD_MODE == "spread":
        # pads must not hammer one HBM row: spread them over the chunk
        pad_fill = np.broadcast_to(
            ((np.arange(slots, dtype=np.int64) * 97) % chunk_rows).astype(np.int16),
            (N_CORES, slots))
        idx16 = pad_fill.copy()
    elif PAD_MODE == "seq":
        # pads sit in contiguous slot runs; sequential addresses turn them
        # into streaming reads (HBM page hits) instead of random reads
        pad_fill = np.broadcast_to(
            (np.arange(slots, dtype=np.int64) % chunk_rows).astype(np.int16),
            (N_CORES, slots))
        idx16 = pad_fill.copy()
    else:
        idx16 = np.zeros((N_CORES, slots), np.int16)
    dstrel = np.full((N_CORES, slots), -1.0, np.float32)
    idx16[core_s, slot] = (g_row[order] - k_s * chunk_rows).astype(np.int16)
    dstrel[core_s, slot] = (dloc[order] % SUB).astype(np.float32)
    if PAD_MODE == "ffill":
        # pads inherit the nearest preceding real index: the duplicate read
        # hits the same (already-open) HBM row, so pads are nearly free
        real = np.zeros((N_CORES, slots), bool)
        real[core_s, slot] = True
        pos = np.where(real, np.arange(slots)[None, :], 0)
        last = np.maximum.accumulate(pos, axis=1)
        idx16 = np.take_along_axis(idx16, last, axis=1)

    # device layouts
    idx_dev = np.stack([_wrap_idx16(idx16[c]) for c in range(N_CORES)])
    dst_dev = np.ascontiguousarray(
        dstrel.reshape(N_CORES, totblk, P).transpose(0, 2, 1)).astype(ml_dtypes.bfloat16)
    iota = np.ascontiguousarray(
        np.broadcast_to(np.arange(SUB, dtype=np.float32), (P, SUB))).astype(ml_dtypes.bfloat16)

    in_maps = []
    for c in range(N_CORES):
        deg_c = np.ones(npc_pad, np.float32)
        deg_c[:npc] = deg[c * npc:(c + 1) * npc]
        deg_wrap = np.ascontiguousarray(deg_c.reshape(jt, P).T)
        deg_rows_c = np.ones(ncols, np.float32)
        deg_rows_c[:npc] = deg[c * npc:(c + 1) * npc]
        x_c = np.zeros((npc_pad, P), np.float32)
        x_c[:npc] = x[c * npc:(c + 1) * npc]
        in_maps.append({
            "x": x_c,
            "w": np.ascontiguousarray(W, dtype=np.float32),
            "bias": np.asarray(b, np.float32).reshape(P, 1),
            "alpha": np.asarray(alpha, np.float32).reshape(P, 1),
            "deg_wrap": deg_wrap,
            "deg_rows": deg_rows_c.reshape(rj, P),
            "idx16": idx_dev[c],
            "dstrel": dst_dev[c],
            "iota": iota,
        })

    meta = dict(npc=npc, npc_pad=npc_pad, nwin=nwin, ncols=ncols, jt=jt, rj=rj,
                nch=nch, chunk_rows=chunk_rows, B=B, Bg=Bg, totblk=totblk,
                nblk_max=max(c[3] for c in calls),
                calls=calls, blk_base=blk_base, nfull=nfull)
    return in_maps, meta


def _build_program(meta):
    npc_pad = meta["npc_pad"]
    nwin = meta["nwin"]
    ncols = meta["ncols"]
    jt = meta["jt"]
    rj = meta["rj"]
    nch = meta["nch"]
    chunk_rows = meta["chunk_rows"]
    Bg = meta["Bg"]
    nblk_max = meta["nblk_max"]
    totblk = meta["totblk"]
    calls = meta["calls"]
    blk_base = meta["blk_base"]
    nfull = meta["nfull"]
    Act = mybir.ActivationFunctionType

    nc = bacc.Bacc("TRN2", target_bir_lowering=False, debug=False,
                   num_devices=N_CORES, num_swdge_queues=NQ)

    x_d = nc.dram_tensor("x", [npc_pad, P], F32, kind="ExternalInput").ap()
    w_d = nc.dram_tensor("w", [P, P], F32, kind="ExternalInput").ap()
    b_d = nc.dram_tensor("bias", [P, 1], F32, kind="ExternalInput").ap()
    alpha_d = nc.dram_tensor("alpha", [P, 1], F32, kind="ExternalInput").ap()
    degw_d = nc.dram_tensor("deg_wrap", [P, jt], F32, kind="ExternalInput").ap()
    degr_d = nc.dram_tensor("deg_rows", [rj, P], F32, kind="ExternalInput").ap()
    idx_d = nc.dram_tensor("idx16", [P, totblk * P // 16], I16,
                           kind="ExternalInput").ap()
    dstr_d = nc.dram_tensor("dstrel", [P, totblk], BF16, kind="ExternalInput").ap()
    iota_d = nc.dram_tensor("iota", [P, SUB], BF16, kind="ExternalInput").ap()
    out_d = nc.dram_tensor("out", [npc_pad, P], F32, kind="ExternalOutput").ap()

    _dbg0 = os.environ.get("GCN_DEBUG", "")
    with tile.TileContext(nc) as tc:
        with tile.ExitStack() as top:
            cpool = top.enter_context(tc.tile_pool(name="const", bufs=1))
            dpool = top.enter_context(tc.tile_pool(name="dram", bufs=1, space="DRAM"))

            # ---- constants ----
            w_f32 = cpool.tile([P, P], F32, name="w_f32")
            nc.sync.dma_start(out=w_f32[:], in_=w_d[:])
            w_bf = cpool.tile([P, P], BF16, name="w_bf")
            nc.vector.tensor_copy(out=w_bf[:], in_=w_f32[:])

            b_sb = cpool.tile([P, 1], F32, name="b_sb")
            nc.sync.dma_start(out=b_sb[:], in_=b_d[:])
            negb_sb = cpool.tile([P, 1], F32, name="negb_sb")
            nc.vector.tensor_scalar_mul(negb_sb[:], b_sb[:], -1.0)
            alpha_sb = cpool.tile([P, 1], F32, name="alpha_sb")
            nc.sync.dma_start(out=alpha_sb[:], in_=alpha_d[:])
            iota_sb = cpool.tile([P, SUB], BF16, name="iota_sb")
            nc.sync.dma_start(out=iota_sb[:], in_=iota_d[:])

            ident_bf = cpool.tile([P, P], BF16, name="ident_bf")
            make_identity(nc, ident_bf[:])
            ident_f32 = cpool.tile([P, P], F32, name="ident_f32")
            make_identity(nc, ident_f32[:])

            # dinv wrapped (per-partition scale for phase 1)
            degw_sb = cpool.tile([P, jt], F32, name="degw_sb")
            nc.sync.dma_start(out=degw_sb[:], in_=degw_d[:])
            dinvw_sb = cpool.tile([P, jt], F32, name="dinvw_sb")
            nc.vector.reciprocal(dinvw_sb[:], degw_sb[:])
            nc.scalar.sqrt(dinvw_sb[:], dinvw_sb[:])

            # dinv natural order -> DRAM (for free-dim broadcast loads)
            degr_sb = cpool.tile([rj, P], F32, name="degr_sb")
            nc.sync.dma_start(out=degr_sb[:], in_=degr_d[:])
            dinvr_sb = cpool.tile([rj, P], F32, name="dinvr_sb")
            nc.vector.reciprocal(dinvr_sb[:], degr_sb[:])
            nc.scalar.sqrt(dinvr_sb[:], dinvr_sb[:])
            dinv_dram = dpool.tile([ncols], F32, name="dinv_dram")
            nc.sync.dma_start(
                out=dinv_dram[:].rearrange("(r k) -> r k", r=rj), in_=dinvr_sb[:])

            # edge tables
            idx_sb = cpool.tile([P, totblk * P // 16], I16, name="idx_sb")
            nc.sync.dma_start(out=idx_sb[:], in_=idx_d[:])
            dstrel_sb = cpool.tile([P, totblk], BF16, name="dstrel_sb")
            nc.sync.dma_start(out=dstrel_sb[:], in_=dstr_d[:])

            g_c = dpool.tile([npc_pad, P], BF16, name="g_c")
            g_full = dpool.tile([nfull, P], BF16, addr_space="Shared", name="g_full")

            # ---- phase 1: g_c = dinv * (x @ W) ----
            if "noph1" not in _dbg0:
                with tile.ExitStack() as ph1:
                    psT_pool = ph1.enter_context(
                        tc.tile_pool(name="ph1psT", bufs=2, space="PSUM"))
                    psH_pool = ph1.enter_context(
                        tc.tile_pool(name="ph1psH", bufs=2, space="PSUM"))
                    ph1_pool = ph1.enter_context(tc.tile_pool(name="ph1sb", bufs=3))
                    x_all = ph1_pool.tile([P, jt, P], BF16, name="x_all", bufs=1)
                    nc.gpsimd.dma_start(
                        out=x_all[:], in_=x_d[:].rearrange("(j p) f -> p j f", p=P))
                    for t in range(jt):
                        xT_ps = psT_pool.tile([P, P], BF16, tag="xT", name="xT_ps")
                        nc.tensor.transpose(out=xT_ps[:], in_=x_all[:, t, :],
                                            identity=ident_bf[:])
                        xT_sb = ph1_pool.tile([P, P], BF16, tag="xTs", name="xT_sb")
                        nc.scalar.activation(out=xT_sb[:], in_=xT_ps[:], func=Act.Copy)
                        h_ps = psH_pool.tile([P, P], F32, tag="h", name="h_ps")
                        nc.tensor.matmul(out=h_ps[:], lhsT=xT_sb[:], rhs=w_bf[:],
                                         start=True, stop=True)
                        g_sb = ph1_pool.tile([P, P], BF16, tag="g", name="g_sb")
                        nc.scalar.activation(out=g_sb[:], in_=h_ps[:], func=Act.Copy,
                                             scale=dinvw_sb[:, t:t + 1])
                        nc.sync.dma_start(out=g_c[t * P:(t + 1) * P, :], in_=g_sb[:])

            # ---- phase 2: replicate g ----
            if "noag" not in os.environ.get("GCN_DEBUG", ""):
                nc.gpsimd.collective_compute(
                    "AllGather",
                    mybir.AluOpType.bypass,
                    replica_groups=[list(range(N_CORES))],
                    ins=[g_c[:].opt()],
                    outs=[g_full[:].opt()],
                )

            # ---- phase 3 ----
            _dbg = os.environ.get("GCN_DEBUG", "")
            if "noph3" in _dbg:
                calls = []
            with tile.ExitStack() as ph3:
                gat_pools = [ph3.enter_context(tc.tile_pool(name=f"gat{q}", bufs=2))
                             for q in range(NQ)]
                oh_pool = ph3.enter_context(tc.tile_pool(name="oh", bufs=2))
                psw_pool = ph3.enter_context(
                    tc.tile_pool(name="psw", bufs=6, space="PSUM"))
                ztr_pool = ph3.enter_context(
                    tc.tile_pool(name="ztr", bufs=2, space="PSUM"))
                epi_pool = ph3.enter_context(tc.tile_pool(name="epi", bufs=2))
                dv_pool = ph3.enter_context(tc.tile_pool(name="dv", bufs=2))

                psw = {}           # window -> psum tile
                done_k = {}        # window -> #chunks matmul'd
                for ci, (kk, wq, blk_start, nblk) in enumerate(calls):
                    q = ci % NQ
                    gat = gat_pools[q].tile([P, nblk_max, P], BF16, tag="gat",
                                            name="gat")
                    if "nogather" in _dbg:
                        nc.vector.memset(gat[:, :nblk, :], 0.0)
                    else:
                        nc.gpsimd.dma_gather(
                            out_ap=gat[:, :nblk, :],
                            in_ap=g_full[kk * chunk_rows:
                                         min((kk + 1) * chunk_rows, nfull), :],
                            idxs_ap=idx_sb[:, blk_start * P // 16:
                                           (blk_start + nblk) * P // 16],
                            num_idxs=nblk * P,
                            num_idxs_reg=nblk * P,
                            elem_size=P,
                            single_packet=SINGLE_PACKET,
                            queue_num=q,
                        )
                    oh = oh_pool.tile([P, nblk_max, SUB], BF16, tag="oh",
                                      name="oh")
                    if "nooh" in _dbg:
                        nc.vector.memset(oh[:, :nblk, :], 0.0)
                    else:
                        nc.vector.tensor_tensor(
                            out=oh[:, :nblk, :],
                            in0=dstrel_sb[:, blk_start:blk_start + nblk]
                                .unsqueeze(2).to_broadcast([P, nblk, SUB]),
                            in1=iota_sb[:].unsqueeze(1).to_broadcast([P, nblk, SUB]),
                            op=mybir.AluOpType.is_equal,
                        )
                    for ww in wq:
                        if ww not in psw:
                            psw[ww] = psw_pool.tile([P, WIN], F32, tag="psw",
                                                    name="psw")
                            nc.vector.memset(psw[ww][:], 0.0)
                            done_k[ww] = 0
                        if "nomm" not in _dbg:
                            for ss in range(NSUB):
                                nb = int(Bg[ww, ss, kk])
                                for i in range(nb):
                                    blk = blk_base[ww, ss, kk] - blk_start + i
                                    nc.tensor.matmul(
                                        out=psw[ww][:, ss * SUB:(ss + 1) * SUB],
                                        lhsT=gat[:, blk, :],
                                        rhs=oh[:, blk, :],
                                        start=False,
                                        stop=(kk == nch - 1 and i == nb - 1),
                                        skip_group_check=True,
                                    )
                        done_k[ww] += 1
                        if done_k[ww] < nch:
                            continue
                        # ---- epilogue for window ww ----
                        pswt = psw.pop(ww)
                        dinv_win = dv_pool.tile([P, WIN], F32, tag="dv",
                                                name="dinv_win")
                        nc.sync.dma_start(
                            out=dinv_win[:],
                            in_=dinv_dram[ww * WIN:(ww + 1) * WIN]
                                .unsqueeze(0).partition_broadcast(P))
                        u = epi_pool.tile([P, WIN], F32, tag="u", name="u")
                        nc.vector.tensor_mul(u[:], pswt[:], dinv_win[:])
                        r = epi_pool.tile([P, WIN], F32, tag="r", name="r")
                        nc.scalar.activation(out=r[:], in_=u[:], func=Act.Relu,
                                             bias=b_sb[:, 0:1], scale=1.0)
                        q = epi_pool.tile([P, WIN], F32, tag="q", name="q")
                        nc.scalar.activation(out=q[:], in_=u[:], func=Act.Relu,
                                             bias=negb_sb[:, 0:1], scale=-1.0)
                        tq = epi_pool.tile([P, WIN], F32, tag="tq", name="tq")
                        nc.scalar.activation(out=tq[:], in_=q[:], func=Act.Copy,
                                             scale=alpha_sb[:, 0:1])
                        z = epi_pool.tile([P, WIN], F32, tag="z", name="z")
                        nc.vector.tensor_sub(z[:], r[:], tq[:])
                        # transpose back and write natural rows
                        for j in range(NSUB):
                            row0 = ww * WIN + j * SUB
                            if row0 >= npc_pad:
                                break
                            zt_ps = ztr_pool.tile([P, P], F32, tag="zt",
                                                  name="zt_ps")
                            nc.tensor.transpose(out=zt_ps[:],
                                                in_=z[:, j * SUB:(j + 1) * SUB],
                                                identity=ident_f32[:])
                            zrow = epi_pool.tile([P, P], F32, tag="zrow",
                                                 name="zrow")
                            nc.scalar.activation(out=zrow[:], in_=zt_ps[:],
                                                 func=Act.Copy)
                            nrows = min(P, npc_pad - row0)
                            nc.sync.dma_start(out=out_d[row0:row0 + nrows, :],
                                              in_=zrow[:nrows, :])

    nc.compile()
    return nc


_CACHE = {}


def kernel(x, edge_index, W, b, alpha):
    x = np.asarray(x)
    edge_index = np.asarray(edge_index)

    in_maps, meta = _host_prep(x, edge_index, np.asarray(W), np.asarray(b),
                               np.asarray(alpha))
    key = (x.shape, edge_index.shape, meta["B"], meta["totblk"])
    if key not in _CACHE:
        _CACHE[key] = _build_program(meta)
    nc = _CACHE[key]

    r = run_bass_kernel_spmd(nc, in_maps, list(range(N_CORES)))
    npc = meta["npc"]
    out = np.concatenate([np.asarray(r.results[c]["out"])[:npc]
                          for c in range(N_CORES)], axis=0)
    return out.astype(np.float32)

